# revision 2
# baseline (speedup 1.0000x reference)
"""Trainium2 Bass kernel v2 for nn_Conv2d_91311004713559 (LUT-conv).

Math: per table t, out[b,t] = a_t + b_t*x0 + c_t*x1 + d_t*x0*x1 (Lagrange
reduction of the K=2 LUT), then tables reduce in groups of TPP=144 per pixel.

v2 split of engines (vs v1 which did everything on DVE):
  - DVE: 4 bf16 elementwise passes (product + 3 coeff multiplies), all 2x mode
  - PE : segmented 144:1 reduction + the 3-way add, via a block-diagonal 0/1
         stationary [128,8] and 28 accumulating matmuls per chunk (the 28th
         folds the per-pixel bias in from a resident moving tile)
  - ACT: PSUM -> SBUF evacuation
Layout: partition = (pixel-in-block a<8) * 16 + (table-slot j<16); free =
(batch, g<9, q<225) + 1 zero pad column per batch so every per-batch DVE op
starts 4B-aligned with even length (keeps 2x mode).

The batch-independent index gather stays host-side: this toolchain rejects
device-side gather (Q7 ext-ISA fails walrus codegen; IndirectCopy fails at
runtime).
"""

import os

import numpy as np
import ml_dtypes

# ---- static problem config (hardcoded per contract) ----
B = 16
IN_CH, OUT_CH = 16, 16
H, W = 32, 32
H_OUT = W_OUT = 30
POS = H_OUT * W_OUT            # 900
TPP = IN_CH * 3 * 3            # 144
T = OUT_CH * POS * TPP         # 2,073,600
N_CORES = 8
T_NC = T // N_CORES            # 259,200 tables / core (= 2 out-channels)
PIX_NC = 2 * POS               # 1800 pixels / core = 8 * 225
QB = 225                       # pixel blocks (q)
AB = 8                         # pixels per block (a) -> partition groups
GB = 9                         # table groups (g)
JB = 16                        # tables per group (j) -> within partition group
FPB = GB * QB                  # 2025 real elems per batch per partition
FPB_PAD = FPB + 1              # 2026, even, keeps per-batch slices 4B-aligned
CHUNK_B = 2                    # batches per streamed chunk
CFREE = CHUNK_B * FPB_PAD      # 4052
NCHUNK = B // CHUNK_B          # 8
XFREE = B * FPB_PAD            # 32416 free elems per partition per stream
# chunks whose x0*x1 product runs on GpSimd instead of DVE (off-loads the
# DVE bottleneck; GpSimd is ~4x slower per element but otherwise idle)
GP_CHUNKS = int(os.environ.get("GP_CHUNKS", "0"))

_NC_CACHE = {}


def _patch_tile_drain_and_waits():
    """This env's walrus accepts at most one semaphore wait per instruction.
    Split Tile's end-of-kernel drain waits, and any other multi-wait
    instruction, onto single-wait InstNoOp's."""
    import concourse.mybir as mybir
    from concourse.tile import TileContext, ScopedClock

    if getattr(TileContext, "_ant_drain_patched", False):
        return

    def _drain_and_barrier(self, tick_clock, wait_clock):
        drain_inst = self.nc.sync.drain()
        wait_clock.add_sem_waits(
            drain_inst.ins, ScopedClock({None: tick_clock.global_clock})
        )
        si = drain_inst.ins.sync_info
        if si is not None and si.on_wait and len(si.on_wait) > 1:
            waits = list(si.on_wait)
            si.on_wait = waits[:1]
            for i in range(1, len(waits)):
                nop = self.nc.sync.nop(nofuse=True)
                nsi = nop.ins.sync_info
                if nsi is None:
                    nop.ins.sync_info = mybir.SyncInfo(
                        on_wait=waits[i : i + 1], on_update=[]
                    )
                else:
                    nsi.on_wait = waits[i : i + 1]
        self.nc.all_engine_barrier()
        popped = self.nc._tile_sem_poison_stack.pop()
        assert popped is self._sem_poison
        self.nc.clear_and_free_semaphores(list(self.sems.allocated().values()))
        self.nc.all_engine_barrier()

    TileContext._drain_and_barrier = _drain_and_barrier
    TileContext._ant_drain_patched = True


def _split_multi_waits(nc):
    import concourse.mybir as mybir

    for f in nc.m.functions:
        for blk in f.blocks:
            il = list(blk.instructions)
            out = []
            changed = False
            for ins in il:
                si = getattr(ins, "sync_info", None)
                if si is not None and si.on_wait and len(si.on_wait) > 1:
                    waits = list(si.on_wait)
                    for i in range(len(waits) - 1):
                        nop = mybir.InstNoOp(name=f"{ins.name}_ws{i}", ins=[], outs=[])
                        nop.engine = ins.engine
                        nop.sync_info = mybir.SyncInfo(
                            on_wait=waits[i : i + 1], on_update=[]
                        )
                        out.append(nop)
                    si.on_wait = waits[-1:]
                    changed = True
                out.append(ins)
            if changed:
                blk.instructions = out


def _build_device_kernel():
    import concourse.bass as bass
    import concourse.mybir as mybir
    from concourse.tile import TileContext

    _patch_tile_drain_and_waits()

    F32 = mybir.dt.float32
    BF16 = mybir.dt.bfloat16
    nc = bass.Bass()

    x0_d = nc.dram_tensor("x0", [128, XFREE], BF16, kind="ExternalInput")
    x1_d = nc.dram_tensor("x1", [128, XFREE], BF16, kind="ExternalInput")
    cb_d = nc.dram_tensor("cb", [128, CFREE], BF16, kind="ExternalInput")
    cc_d = nc.dram_tensor("cc", [128, CFREE], BF16, kind="ExternalInput")
    cd_d = nc.dram_tensor("cd", [128, CFREE], BF16, kind="ExternalInput")
    sel_d = nc.dram_tensor("sel", [128, AB], BF16, kind="ExternalInput")
    bm_d = nc.dram_tensor("bm", [128, CHUNK_B * QB], BF16, kind="ExternalInput")
    out_d = nc.dram_tensor("out", [AB, B * QB], F32, kind="ExternalOutput")

    mult = mybir.AluOpType.mult

    with TileContext(nc) as tc:
        with (
            tc.tile_pool(name="coef", bufs=1) as cpool,
            tc.tile_pool(name="x0p", bufs=4) as x0pool,
            tc.tile_pool(name="x1p", bufs=4) as x1pool,
            tc.tile_pool(name="mp", bufs=4) as mpool,
            tc.tile_pool(name="psum", bufs=3, space="PSUM") as ppool,
            tc.tile_pool(name="outp", bufs=1) as opool,
        ):
            # issue chunk 0's data DMAs FIRST: the HWDGE ring drains in FIFO
            # order, and the first TT (x0*x1) needs no coefficients — this
            # pulls the DVE start-up in by ~16us.
            x0t0 = x0pool.tile([128, CFREE], BF16, tag="x0t")
            nc.sync.dma_start(x0t0[:], x0_d[:, 0:CFREE])
            x1t0 = x1pool.tile([128, CFREE], BF16, tag="x1t")
            nc.sync.dma_start(x1t0[:], x1_d[:, 0:CFREE])

            cbt = cpool.tile([128, CFREE], BF16)
            nc.scalar.dma_start(cbt[:], cb_d[:])
            cct = cpool.tile([128, CFREE], BF16)
            nc.scalar.dma_start(cct[:], cc_d[:])
            cdt = cpool.tile([128, CFREE], BF16)
            nc.scalar.dma_start(cdt[:], cd_d[:])
            selt = cpool.tile([128, AB], BF16)
            nc.scalar.dma_start(selt[:], sel_d[:])
            bmt = cpool.tile([128, CHUNK_B * QB], BF16)
            nc.scalar.dma_start(bmt[:], bm_d[:])
            out_sb = opool.tile([AB, B * QB], F32)

            bm_v = bmt[:].rearrange("p (b q) -> p b q", b=CHUNK_B)

            for c in range(NCHUNK):
                sl = slice(c * CFREE, (c + 1) * CFREE)
                if c == 0:
                    x0t, x1t = x0t0, x1t0
                else:
                    x0t = x0pool.tile([128, CFREE], BF16, tag="x0t")
                    nc.sync.dma_start(x0t[:], x0_d[:, sl])
                    x1t = x1pool.tile([128, CFREE], BF16, tag="x1t")
                    nc.sync.dma_start(x1t[:], x1_d[:, sl])
                mt = mpool.tile([128, CFREE], BF16)
                # full-chunk elementwise, bf16 2x mode (coeff tiles are
                # pre-widened x CHUNK_B on host so everything is contiguous)
                prod_eng = nc.gpsimd if (c % NCHUNK) < GP_CHUNKS else nc.vector
                prod_eng.tensor_tensor(mt[:], x0t[:], x1t[:], op=mult)
                nc.vector.tensor_tensor(x0t[:], x0t[:], cbt[:], op=mult)
                nc.vector.tensor_tensor(x1t[:], x1t[:], cct[:], op=mult)
                nc.vector.tensor_tensor(mt[:], mt[:], cdt[:], op=mult)
                # PE: segmented reduce over j (partition blocks of 16) and g,
                # summing the three streams + bias into one PSUM tile.
                pt = ppool.tile([AB, CHUNK_B * QB], F32)
                pt_v = pt[:].rearrange("p (b q) -> p b q", b=CHUNK_B)
                mms = []
                for s in (x0t, x1t, mt):
                    sv = s[:].rearrange("p (b f) -> p b f", b=CHUNK_B)
                    for g in range(GB):
                        mms.append(sv[:, :, g * QB : (g + 1) * QB])
                mms.append(bm_v)
                for i, rhs in enumerate(mms):
                    nc.tensor.matmul(
                        pt_v,
                        selt[:],
                        rhs,
                        start=(i == 0),
                        stop=(i == len(mms) - 1),
                    )
                nc.scalar.copy(
                    out_sb[:, c * CHUNK_B * QB : (c + 1) * CHUNK_B * QB], pt[:]
                )
            nc.sync.dma_start(out_d[:], out_sb[:])

    _split_multi_waits(nc)
    return nc


def _marshal_tables(arr):
    """[..., T_NC] (per-core table axis, p-major then r) ->
    [..., 128, FPB] with partition = a*16+j, free = (g, q)."""
    v = arr.reshape(arr.shape[:-1] + (QB, AB, GB, JB))
    # [..., q, a, g, j] -> [..., a, j, g, q]
    nd = v.ndim
    perm = tuple(range(nd - 4)) + (nd - 3, nd - 1, nd - 2, nd - 4)
    v = v.transpose(perm)
    return v.reshape(arr.shape[:-1] + (128, FPB))


def kernel(x, input_mask, weight):
    from concourse.bass_utils import run_bass_kernel_spmd

    x = np.asarray(x, dtype=np.float32)
    input_mask = np.asarray(input_mask)
    weight = np.asarray(weight, dtype=np.float32)

    # ---- host: batch-independent gather + coeff transform + marshaling ----
    lin = (
        input_mask[:, 0].astype(np.int64) * (H * W)
        + input_mask[:, 1].astype(np.int64) * W
        + input_mask[:, 2].astype(np.int64)
    )
    flat = x.reshape(B, IN_CH * H * W)
    gathered = flat[:, lin]                      # [B, 2T]
    x0 = gathered[:, 0::2]                       # [B, T]
    x1 = gathered[:, 1::2]

    w0, w1, w2, w3 = weight[:, 0], weight[:, 1], weight[:, 2], weight[:, 3]
    ca = 0.25 * (w0 + w1 + w2 + w3)
    cb = 0.25 * (-w0 + w1 - w2 + w3)
    cc = 0.25 * (-w0 - w1 + w2 + w3)
    cd = 0.25 * (w0 - w1 - w2 + w3)

    bf = ml_dtypes.bfloat16
    sel = np.zeros((128, AB), dtype=bf)
    for a in range(AB):
        sel[a * JB : (a + 1) * JB, a] = 1.0

    x0_s = x0.reshape(B, N_CORES, T_NC)
    x1_s = x1.reshape(B, N_CORES, T_NC)
    cb_s = cb.reshape(N_CORES, T_NC)
    cc_s = cc.reshape(N_CORES, T_NC)
    cd_s = cd.reshape(N_CORES, T_NC)
    ca_s = ca.reshape(N_CORES, T_NC)

    in_maps = []
    for n in range(N_CORES):
        def xlay(a_s):
            v = _marshal_tables(a_s[:, n])       # [B, 128, FPB]
            vp = np.zeros((B, 128, FPB_PAD), dtype=bf)
            vp[:, :, :FPB] = v
            # free dim = (b, fpb_pad)
            return np.ascontiguousarray(
                vp.transpose(1, 0, 2).reshape(128, XFREE)
            )

        def clay(c_t):
            v = _marshal_tables(c_t[n])          # [128, FPB]
            vp = np.zeros((128, FPB_PAD), dtype=bf)
            vp[:, :FPB] = v
            return np.ascontiguousarray(np.tile(vp, (1, CHUNK_B)))

        # per-pixel bias, laid out as a moving tile: bm[a*16+j, (b,q)] =
        # bias[8q+a]/16 (j-copies sum back to bias via the selector matmul)
        bias = (
            ca_s[n]
            .reshape(QB, AB, TPP)
            .sum(axis=-1, dtype=np.float64)
            .astype(np.float32)
        )                                        # [q, a]
        bm = np.zeros((128, CHUNK_B * QB), dtype=bf)
        bq = np.repeat((bias.T / JB)[:, None, :], CHUNK_B, axis=1)  # [a,b,q]
        for a in range(AB):
            bm[a * JB : (a + 1) * JB, :] = bq[a].reshape(1, CHUNK_B * QB)

        in_maps.append(
            {
                "x0": xlay(x0_s),
                "x1": xlay(x1_s),
                "cb": clay(cb_s),
                "cc": clay(cc_s),
                "cd": clay(cd_s),
                "sel": sel,
                "bm": bm,
            }
        )

    key = ("nc", GP_CHUNKS)
    if key not in _NC_CACHE:
        _NC_CACHE[key] = _build_device_kernel()
    nc = _NC_CACHE[key]

    res = run_bass_kernel_spmd(nc, in_maps, core_ids=list(range(N_CORES)))

    # ---- unshard: out_dev[a, b*QB+q] = pixel (8q+a) of batch b ----
    out = np.empty((B, OUT_CH, H_OUT, W_OUT), dtype=np.float32)
    for n in range(N_CORES):
        o = np.asarray(res.results[n]["out"], dtype=np.float32)  # [8, B*QB]
        o = o.reshape(AB, B, QB).transpose(1, 2, 0).reshape(B, PIX_NC)
        pix = o.reshape(B, 2, POS)
        out[:, 2 * n] = pix[:, 0].reshape(B, H_OUT, W_OUT)
        out[:, 2 * n + 1] = pix[:, 1].reshape(B, H_OUT, W_OUT)
    return out
